# revision 14
# baseline (speedup 1.0000x reference)
"""AxialAttention Trainium2 kernel: 8-core data-parallel over batch.

Strategy (validated in numpy mock, rel-l2 ~7e-4 vs reference):
- All BN affines folded on host into projection weights / RPE tables / output bias.
- Per core: 65 items (B padded 516->520). For_i loop over items.
- fp16 matmuls (PE full rate), fp32 PSUM/softmax internals.
- RPE "skew" terms via DRAM round-trips with pitch-256 flat layouts (collision-free).
- kt table column-reversed on host so both skew reads are positive-stride.
- 129x129-ish transposes via DMA-transpose (xbar) in 128x128 + 16x128 blocks.
"""
import numpy as np

EPS = 1e-3
H, DK, DV, L, C = 8, 64, 128, 129, 512
NC = 8
NI = 65               # items per core (520/8)
BS = 33280            # per-head flat skew-buffer elements
HS2 = 272             # fp16 per-head stride (544B, 32B-aligned)
HS4 = 136             # fp32 per-head stride (544B)

_CACHE = {}


def _host_prep(inputs):
    g = lambda k: np.asarray(inputs[k], np.float64)
    s_qkv = g("gamma_qkv") / np.sqrt(g("var_qkv") + EPS)
    t_qkv = g("beta_qkv") - g("mean_qkv") * s_qkv
    s_sim = g("gamma_sim") / np.sqrt(g("var_sim") + EPS)
    s_out = g("gamma_out") / np.sqrt(g("var_out") + EPS)
    t_out = g("beta_out") - g("mean_out") * s_out
    W = g("qkv_kernel")
    s0 = np.repeat(s_sim[0], DK)
    Wq = W[:, :512] * s_qkv[:512] * s0
    tq = t_qkv[:512] * s0
    Wk = W[:, 512:1024] * s_qkv[512:1024]
    tk = t_qkv[512:1024].copy()
    qt = g("query_rpe_table"); kt = g("key_rpe_table"); vt = g("value_rpe_table")
    s_v = s_qkv[1024:].reshape(H, DV)
    t_v = t_qkv[1024:].reshape(H, DV)
    Wv = W[:, 1024:] * (s_v * s_out[0]).reshape(-1)
    t_prime = t_v * s_out[0] + t_out[0] + t_out[1]          # [H, DV]

    wqk = np.concatenate([Wq, Wk], 1).reshape(4, 128, 1024).astype(np.float16)
    wv = Wv.reshape(4, 128, 1024).astype(np.float16)
    tqk = np.concatenate([tq, tk]).reshape(16, 64).T.astype(np.float32)  # [64, 16]
    # tables: [64, H, 2, HS2]; kt column-REVERSED (j' = 256 - j)
    qktab = np.zeros((64, H, 2, HS2), np.float16)
    for h in range(H):
        qktab[:, h, 0, :257] = (qt.T * (s_sim[1, h] / s_sim[0, h])).astype(np.float16)
        qktab[:, h, 1, :257] = (kt.T * s_sim[2, h])[:, ::-1].astype(np.float16)
    vt_s = np.stack([vt * s_out[1, h] for h in range(H)])    # [H, 257, DV]
    vt0 = vt_s[:, 0:128].transpose(1, 0, 2).reshape(128, H * DV).astype(np.float16)
    vt1 = vt_s[:, 128:256].transpose(1, 0, 2).reshape(128, H * DV).astype(np.float16)
    vt2 = vt_s[:, 256].reshape(1, H * DV).astype(np.float16)
    tp = t_prime.reshape(1, H * DV).astype(np.float16)       # [1, H*DV]
    return dict(wqk=wqk, wv=wv, tqk=tqk, qktab=qktab, vt0=vt0, vt1=vt1, vt2=vt2, tp=tp)


def _build(ni=NI, dbg=False):
    import concourse.bass as bass
    import concourse.bacc as bacc
    import concourse.mybir as mybir
    import concourse.tile as tile
    from concourse.tile import add_dep_helper
    F16, F32 = mybir.dt.float16, mybir.dt.float32
    AF = mybir.ActivationFunctionType
    ALU = mybir.AluOpType
    AX = mybir.AxisListType

    nc = bacc.Bacc("TRN2", target_bir_lowering=False, debug=False, num_devices=NC)
    x_d = nc.dram_tensor("x", [ni, L, C], F16, kind="ExternalInput")
    wqk_d = nc.dram_tensor("wqk", [4, 128, 1024], F16, kind="ExternalInput")
    wv_d = nc.dram_tensor("wv", [4, 128, 1024], F16, kind="ExternalInput")
    tqk_d = nc.dram_tensor("tqk", [64, 16], F32, kind="ExternalInput")
    qktab_d = nc.dram_tensor("qktab", [64, H, 2, HS2], F16, kind="ExternalInput")
    vt0_d = nc.dram_tensor("vt0", [128, 1024], F16, kind="ExternalInput")
    vt1_d = nc.dram_tensor("vt1", [128, 1024], F16, kind="ExternalInput")
    vt2_d = nc.dram_tensor("vt2", [1, 1024], F16, kind="ExternalInput")
    tp_d = nc.dram_tensor("tp", [1, H * DV], F16, kind="ExternalInput")
    out_d = nc.dram_tensor("out", [ni, L, 1024], F16, kind="ExternalOutput")
    if dbg:
        dbg_qkT = nc.dram_tensor("dbg_qkT", [64, 16, HS4], F16, kind="ExternalOutput")
        dbg_v = nc.dram_tensor("dbg_v", [128, 1024], F16, kind="ExternalOutput")
        dbg_sims = nc.dram_tensor("dbg_sims", [128, 8, HS4], F32, kind="ExternalOutput")
        dbg_s2 = nc.dram_tensor("dbg_s2", [128, 8, HS4], F16, kind="ExternalOutput")
        dbg_s3T = nc.dram_tensor("dbg_s3T", [128, 8, 144], F16, kind="ExternalOutput")
        dbg_w = nc.dram_tensor("dbg_w", [128, 8, HS2], F16, kind="ExternalOutput")
        dbg_xT = nc.dram_tensor("dbg_xT", [128, 4, 144], F16, kind="ExternalOutput")
    def dap(t, off, ap):
        a = t[:] if isinstance(t, bass.AP) or not hasattr(t, "ap") or not callable(t.ap) else t.ap()
        if not isinstance(a, bass.AP):
            a = t[:]
        return bass.AP(tensor=a.tensor, offset=a.offset + off, ap=ap)

    with tile.TileContext(nc) as tc:
        with (tc.tile_pool(name="st", bufs=1) as st,
              tc.tile_pool(name="wk", bufs=1) as wk,
              tc.tile_pool(name="psA", bufs=3, space="PSUM") as psA,
              tc.tile_pool(name="psR", bufs=2, space="PSUM") as psR,
              tc.tile_pool(name="psV", bufs=2, space="PSUM") as psV,
              tc.tile_pool(name="psRv", bufs=1, space="PSUM") as psRv,
              tc.tile_pool(name="dsc", bufs=1, space="DRAM") as dsc):
            aq_d = dsc.tile([H, BS], F16)
            bk_d = dsc.tile([H, BS], F16)
            f_d = dsc.tile([H, BS], F16)
            # ---- statics ----
            wqk_sb = st.tile([128, 4, 1024], F16)
            nc.sync.dma_start(out=wqk_sb[:], in_=wqk_d.ap()[:].rearrange("c p n -> p c n"))
            wv_sb = st.tile([128, 4, 1024], F16)
            nc.sync.dma_start(out=wv_sb[:], in_=wv_d.ap()[:].rearrange("c p n -> p c n"))
            tqk_sb = st.tile([64, 16], F32)
            nc.sync.dma_start(out=tqk_sb[:], in_=tqk_d.ap()[:])
            qktab_sb = st.tile([64, H, 2, HS2], F16)
            nc.sync.dma_start(out=qktab_sb[:], in_=qktab_d.ap()[:])
            vt0_sb = st.tile([128, 1024], F16)
            nc.sync.dma_start(out=vt0_sb[:], in_=vt0_d.ap()[:])
            vt1_sb = st.tile([128, 1024], F16)
            nc.sync.dma_start(out=vt1_sb[:], in_=vt1_d.ap()[:])
            vt2_sb = st.tile([1, 1024], F16)
            nc.sync.dma_start(out=vt2_sb[:], in_=vt2_d.ap()[:])
            tp_sb = st.tile([1, 8, 128], F16)
            nc.sync.dma_start(out=tp_sb[0:1, :, :], in_=tp_d.ap()[:].rearrange("a (h d) -> a h d", h=8))
            ones_sb = st.tile([1, 128], F16)
            nc.vector.memset(ones_sb[:], 1.0)
            # zero-fill the W2 skew buffer (gaps must stay zero forever)
            zf = st.tile([128, 2080], F16)
            nc.vector.memset(zf[:], 0.0)
            nc.sync.dma_start(out=dap(f_d, 0, [[2080, 128], [1, 2080]]), in_=zf[:])
            tc.strict_bb_all_engine_barrier()

            with tc.For_i(0, NI) as it:
                # ---- P0: load x (fp16), transpose ----
                xh = wk.tile([128, 512], F16)
                xrh = wk.tile([16, 512], F16)
                nc.sync.dma_start(out=xh[:], in_=x_d.ap()[bass.ds(it, 1), 0:128, :])
                nc.sync.dma_start(out=xrh[0:1, :], in_=x_d.ap()[bass.ds(it, 1), 128:129, :])
                xT = wk.tile([128, 4, 144], F16)
                for c in range(4):
                    nc.sync.dma_start(out=xT[:, c, 0:128], in_=xh[:, c * 128:(c + 1) * 128], transpose=True)
                    nc.sync.dma_start(out=xT[:, c, 128:144], in_=xrh[:, c * 128:(c + 1) * 128], transpose=True)
                # ---- P1: qk projection -> qkT [64 part, 16 head-chunks, 136] f16 ----
                qkT = wk.tile([64, 16, HS4], F16)
                for hc in range(16):
                    ps = psA.tile([128, 257], F32, tag="psA")
                    for c in range(4):
                        nc.tensor.matmul(ps[0:64, 0:129], wqk_sb[:, c, hc * 64:(hc + 1) * 64],
                                         xT[:, c, 0:129], start=(c == 0), stop=(c == 3))
                    nc.scalar.activation(out=qkT[:, hc, 0:129], in_=ps[0:64, 0:129],
                                         func=AF.Identity, bias=tqk_sb[:, hc:hc + 1], scale=1.0)
                # ---- P2: v projection -> v_sb [m part, 1024] f16 ----
                v_sb = wk.tile([128, 1024], F16)
                vr_sb = wk.tile([1, 1024], F16)
                for nch in range(2):
                    ps = psV.tile([128, 512], F32, tag="psV")
                    for c in range(4):
                        nc.tensor.matmul(ps[:], xT[:, c, 0:128], wv_sb[:, c, nch * 512:(nch + 1) * 512],
                                         start=(c == 0), stop=(c == 3))
                    nc.scalar.activation(out=v_sb[:, nch * 512:(nch + 1) * 512], in_=ps[:],
                                         func=AF.Copy, bias=0.0, scale=1.0)
                    psr = psRv.tile([1, 512], F32, tag="psRv")
                    for c in range(4):
                        nc.tensor.matmul(psr[:], xT[:, c, 128:129], wv_sb[:, c, nch * 512:(nch + 1) * 512],
                                         start=(c == 0), stop=(c == 3))
                    nc.scalar.activation(out=vr_sb[:, nch * 512:(nch + 1) * 512], in_=psr[:],
                                         func=AF.Copy, bias=0.0, scale=1.0)

                # ---- P3: per head sims matmuls; evict; Aq/Bk ----
                sims = wk.tile([128, 8, HS4], F32)
                simsr = wk.tile([1, 8, HS4], F32)
                aq_sb = wk.tile([128, 8, HS2], F16)
                aqr_sb = wk.tile([1, 8, HS2], F16)
                bk_sb = wk.tile([128, 8, HS2], F16)
                bkr_sb = wk.tile([16, 8, HS2], F16)
                for h in range(8):
                    qT = qkT[:, h, :]
                    kT = qkT[:, 8 + h, :]
                    cs = psA.tile([128, 257], F32, tag="psA")
                    nc.tensor.matmul(cs[:, 0:129], qT[:, 0:128], kT[:, 0:129], start=True, stop=True)
                    nc.vector.tensor_copy(out=sims[:, h, 0:129], in_=cs[:, 0:129])
                    csr = psR.tile([1, 257], F32, tag="psR")
                    nc.tensor.matmul(csr[:, 0:129], qT[:, 128:129], kT[:, 0:129], start=True, stop=True)
                    nc.vector.tensor_copy(out=simsr[:, h, 0:129], in_=csr[:, 0:129])
                    aq = psA.tile([128, 257], F32, tag="psA")
                    nc.tensor.matmul(aq[:, 0:257], qT[:, 0:128], qktab_sb[:, h, 0, 0:257], start=True, stop=True)
                    nc.scalar.activation(out=aq_sb[:, h, 0:257], in_=aq[:, 0:257], func=AF.Copy, bias=0.0, scale=1.0)
                    aqr = psR.tile([1, 257], F32, tag="psR")
                    nc.tensor.matmul(aqr[:, 0:257], qT[:, 128:129], qktab_sb[:, h, 0, 0:257], start=True, stop=True)
                    nc.scalar.activation(out=aqr_sb[:, h, 0:257], in_=aqr[:, 0:257], func=AF.Copy, bias=0.0, scale=1.0)
                    bk = psA.tile([128, 257], F32, tag="psA")
                    nc.tensor.matmul(bk[:, 0:257], kT[:, 0:128], qktab_sb[:, h, 1, 0:257], start=True, stop=True)
                    nc.scalar.activation(out=bk_sb[:, h, 0:257], in_=bk[:, 0:257], func=AF.Copy, bias=0.0, scale=1.0)
                    bkr = psR.tile([1, 257], F32, tag="psR")
                    nc.tensor.matmul(bkr[:, 0:257], kT[:, 128:129], qktab_sb[:, h, 1, 0:257], start=True, stop=True)
                    nc.scalar.activation(out=bkr_sb[0:1, h, 0:257], in_=bkr[:, 0:257], func=AF.Copy, bias=0.0, scale=1.0)

                # ---- P4: skew-buffer writes (batched over heads) ----
                # Aq: row0 full / rows1-127 [1:256] / row128 full ; addr = 256*l + j
                aw = [nc.sync.dma_start(out=dap(aq_d, 0, [[1, 1], [BS, 8], [1, 257]]), in_=aq_sb[0:1, :, 0:257]),
                      nc.sync.dma_start(out=dap(aq_d, 257, [[256, 127], [BS, 8], [1, 255]]), in_=aq_sb[1:128, :, 1:256]),
                      nc.sync.dma_start(out=dap(aq_d, 32768, [[1, 1], [BS, 8], [1, 257]]), in_=aqr_sb[0:1, :, 0:257])]
                # Bk(rev): row0 full / rows1-127 [1:256] / row128 [0:256] ; addr = 256*m + j'
                bw = [nc.sync.dma_start(out=dap(bk_d, 0, [[1, 1], [BS, 8], [1, 257]]), in_=bk_sb[0:1, :, 0:257]),
                      nc.sync.dma_start(out=dap(bk_d, 257, [[256, 127], [BS, 8], [1, 255]]), in_=bk_sb[1:128, :, 1:256]),
                      nc.sync.dma_start(out=dap(bk_d, 32768, [[1, 1], [BS, 8], [1, 256]]), in_=bkr_sb[0:1, :, 0:256])]
                # ---- P5: skew reads ----
                s2m = wk.tile([128, 8, HS4], F16)   # sim2 [l, h, m]
                s2r = wk.tile([1, 8, HS4], F16)
                s3a = wk.tile([128, 8, HS2], F16)   # sim3T [m, h, l]: addr = 255*m + l + 128
                s3r = wk.tile([16, 8, HS2], F16)
                rds = [nc.sync.dma_start(out=s2m[:, :, 0:129], in_=dap(aq_d, 128, [[255, 128], [BS, 8], [1, 129]])),
                       nc.sync.dma_start(out=s2r[:, :, 0:129], in_=dap(aq_d, 32768, [[1, 1], [BS, 8], [1, 129]])),
                       nc.sync.dma_start(out=s3a[:, :, 0:129], in_=dap(bk_d, 128, [[255, 128], [BS, 8], [1, 129]])),
                       nc.sync.dma_start(out=s3r[0:1, :, 0:129], in_=dap(bk_d, 255 * 128 + 128, [[1, 1], [BS, 8], [1, 129]]))]
                for rd in rds[:2]:
                    for wr in aw:
                        add_dep_helper(rd.ins, wr.ins, sync=True, reason="aq skew RAW")
                for rd in rds[2:]:
                    for wr in bw:
                        add_dep_helper(rd.ins, wr.ins, sync=True, reason="bk skew RAW")
                # ---- P6/P7: per head: transpose sim3T, assemble, softmax ----
                s3T = wk.tile([128, 8, 144], F16)
                s3x = wk.tile([128, 8, 144], F16)
                e32 = wk.tile([128, 8, HS4], F32)
                er32 = wk.tile([1, 8, HS4], F32)
                sums = wk.tile([128, 8], F32)
                sumsr = wk.tile([1, 8], F32)
                w16 = wk.tile([128, 8, HS2], F16)
                wr16 = wk.tile([16, 8, HS2], F16)
                for h in range(8):
                    nc.sync.dma_start(out=s3T[:, h, 0:128], in_=s3a[:, h, 0:128], transpose=True)
                    nc.sync.dma_start(out=s3T[:, h, 128:144], in_=s3r[:, h, 0:128], transpose=True)
                    nc.sync.dma_start(out=s3x[:, h, 0:128], in_=s3a[:, h, 128:256], transpose=True)
                for h in range(8):
                    nc.vector.tensor_add(sims[:, h, 0:129], sims[:, h, 0:129], s2m[:, h, 0:129])
                    nc.vector.tensor_add(sims[:, h, 0:129], sims[:, h, 0:129], s3T[:, h, 0:129])
                    nc.vector.tensor_add(simsr[:, h, 0:129], simsr[:, h, 0:129], s2r[:, h, 0:129])
                    nc.vector.tensor_add(simsr[:, h, 0:128], simsr[:, h, 0:128], s3x[0:1, h, 0:128])
                    nc.vector.tensor_add(simsr[:, h, 128:129], simsr[:, h, 128:129], s3r[0:1, h, 128:129])
                    nc.scalar.activation(out=e32[:, h, 0:129], in_=sims[:, h, 0:129], func=AF.Exp, bias=0.0, scale=1.0)
                    nc.scalar.activation(out=er32[:, h, 0:129], in_=simsr[:, h, 0:129], func=AF.Exp, bias=0.0, scale=1.0)
                    nc.vector.reduce_sum(sums[:, h:h + 1], e32[:, h, 0:129], axis=AX.X)
                    nc.vector.reduce_sum(sumsr[:, h:h + 1], er32[:, h, 0:129], axis=AX.X)
                recips = wk.tile([128, 8], F32)
                recipsr = wk.tile([1, 8], F32)
                nc.vector.reciprocal(out=recips[:], in_=sums[:])
                nc.vector.reciprocal(out=recipsr[:], in_=sumsr[:])
                for h in range(8):
                    nc.vector.tensor_scalar(out=w16[:, h, 0:129], in0=e32[:, h, 0:129],
                                            scalar1=recips[:, h:h + 1], scalar2=None, op0=ALU.mult)
                    nc.vector.tensor_scalar(out=wr16[0:1, h, 0:129], in0=er32[:, h, 0:129],
                                            scalar1=recipsr[:, h:h + 1], scalar2=None, op0=ALU.mult)
                # ---- P8: transposes of w ----
                wT = wk.tile([128, 8, 144], F16)
                wTx = wk.tile([128, 8, 144], F16)
                for h in range(8):
                    nc.sync.dma_start(out=wT[:, h, 0:128], in_=w16[:, h, 0:128], transpose=True)
                    nc.sync.dma_start(out=wT[:, h, 128:144], in_=wr16[:, h, 0:128], transpose=True)
                    nc.sync.dma_start(out=wTx[:, h, 0:128], in_=w16[:, h, 128:256], transpose=True)
                # ---- P9: W2 writes; P10: W2 reads; P11: W2 transposes ----
                fw = [nc.sync.dma_start(out=dap(f_d, 128, [[256, 128], [BS, 8], [1, 129]]), in_=w16[:, :, 0:129]),
                      nc.sync.dma_start(out=dap(f_d, 32896, [[1, 1], [BS, 8], [1, 129]]), in_=wr16[0:1, :, 0:129])]
                w2_sb = wk.tile([128, 8, HS2], F16)
                w2r_sb = wk.tile([16, 8, HS2], F16)
                frd = [nc.sync.dma_start(out=w2_sb[:, :, 0:257], in_=dap(f_d, 0, [[257, 128], [BS, 8], [1, 257]])),
                       nc.sync.dma_start(out=w2r_sb[0:1, :, 0:257], in_=dap(f_d, 32896, [[1, 1], [BS, 8], [1, 257]]))]
                for rd in frd:
                    for wr in fw:
                        add_dep_helper(rd.ins, wr.ins, sync=True, reason="w2 skew RAW")
                w2T0 = wk.tile([128, 8, 144], F16)
                w2T1 = wk.tile([128, 8, 144], F16)
                for h in range(8):
                    nc.sync.dma_start(out=w2T0[:, h, 0:128], in_=w2_sb[:, h, 0:128], transpose=True)
                    nc.sync.dma_start(out=w2T1[:, h, 0:128], in_=w2_sb[:, h, 128:256], transpose=True)
                    nc.sync.dma_start(out=w2T0[:, h, 128:144], in_=w2r_sb[:, h, 0:128], transpose=True)
                    nc.sync.dma_start(out=w2T1[:, h, 128:144], in_=w2r_sb[:, h, 128:256], transpose=True)
                # ---- P12: retrieval matmuls + rank-1 fix + evict ----
                out_sb = wk.tile([128, 1024], F16)
                outr_sb = wk.tile([1, 1024], F16)
                for h in range(8):
                    rp = psA.tile([128, 257], F32, tag="psA")
                    nc.tensor.matmul(rp[:, 0:128], wT[:, h, 0:128], v_sb[:, h * 128:(h + 1) * 128], start=True, stop=False)
                    nc.tensor.matmul(rp[:, 0:128], wTx[0:1, h, 0:128], vr_sb[:, h * 128:(h + 1) * 128], start=False, stop=False)
                    nc.tensor.matmul(rp[:, 0:128], ones_sb[:, 0:128], tp_sb[0:1, h, :], start=False, stop=False)
                    nc.tensor.matmul(rp[:, 0:128], w2T0[:, h, 0:128], vt0_sb[:, h * 128:(h + 1) * 128], start=False, stop=False)
                    nc.tensor.matmul(rp[:, 0:128], w2T1[:, h, 0:128], vt1_sb[:, h * 128:(h + 1) * 128], start=False, stop=True)
                    rk = wk.tile([1, 128], F32, tag="rk1")
                    nc.vector.tensor_scalar(out=rk[:], in0=vt2_sb[:, h * 128:(h + 1) * 128],
                                            scalar1=e32[0:1, h, 128:129], scalar2=recips[0:1, h:h + 1],
                                            op0=ALU.mult, op1=ALU.mult)
                    nc.vector.tensor_add(rp[0:1, 0:128], rp[0:1, 0:128], rk[:])
                    nc.scalar.activation(out=out_sb[:, h * 128:(h + 1) * 128], in_=rp[:, 0:128],
                                         func=AF.Copy, bias=0.0, scale=1.0)
                    rpr = psR.tile([1, 257], F32, tag="psR")
                    nc.tensor.matmul(rpr[:, 0:128], wT[:, h, 128:129], v_sb[:, h * 128:(h + 1) * 128], start=True, stop=False)
                    nc.tensor.matmul(rpr[:, 0:128], wr16[0:1, h, 128:129], vr_sb[:, h * 128:(h + 1) * 128], start=False, stop=False)
                    nc.tensor.matmul(rpr[:, 0:128], ones_sb[:, 0:1], tp_sb[0:1, h, :], start=False, stop=False)
                    nc.tensor.matmul(rpr[:, 0:128], w2T0[:, h, 128:129], vt0_sb[:, h * 128:(h + 1) * 128], start=False, stop=False)
                    nc.tensor.matmul(rpr[:, 0:128], w2T1[:, h, 128:129], vt1_sb[:, h * 128:(h + 1) * 128], start=False, stop=True)
                    nc.scalar.activation(out=outr_sb[:, h * 128:(h + 1) * 128], in_=rpr[:, 0:128],
                                         func=AF.Copy, bias=0.0, scale=1.0)
                # ---- P13: store ----
                nc.sync.dma_start(out=out_d.ap()[bass.ds(it, 1), 0:128, :], in_=out_sb[:])
                nc.sync.dma_start(out=out_d.ap()[bass.ds(it, 1), 128:129, :], in_=outr_sb[:])
                if dbg:
                    nc.sync.dma_start(out=dbg_qkT.ap()[:], in_=qkT[:])
                    nc.sync.dma_start(out=dbg_v.ap()[:], in_=v_sb[:])
                    nc.sync.dma_start(out=dbg_sims.ap()[:], in_=sims[:])
                    nc.sync.dma_start(out=dbg_s2.ap()[:], in_=s2m[:])
                    nc.sync.dma_start(out=dbg_s3T.ap()[:], in_=s3T[:])
                    nc.sync.dma_start(out=dbg_w.ap()[:], in_=w16[:])
                    nc.sync.dma_start(out=dbg_xT.ap()[:], in_=xT[:])
    nc.compile()
    return nc


def _run_cached(nc, maps):
    """Like run_bass_kernel_spmd's axon path, but the jit closure is built once."""
    import jax
    import jax.numpy as jnp
    from jax.sharding import Mesh, PartitionSpec
    from jax.experimental.shard_map import shard_map
    import concourse.mybir as mybir
    from concourse import bass2jax

    if "jit" not in _CACHE:
        bass2jax.install_neuronx_cc_hook()
        in_names, out_names, out_avals = [], [], []
        partition_name = nc.partition_id_tensor.name if nc.partition_id_tensor else None
        for alloc in nc.m.functions[0].allocations:
            if not isinstance(alloc, mybir.MemoryLocationSet):
                continue
            name = alloc.memorylocations[0].name
            if alloc.kind == "ExternalInput":
                if name != partition_name:
                    in_names.append(name)
            elif alloc.kind == "ExternalOutput":
                shape = tuple(alloc.tensor_shape)
                out_names.append(name)
                out_avals.append(jax.core.ShapedArray(shape, mybir.dt.np(alloc.dtype)))
        n_params = len(in_names)
        all_names = in_names + out_names + ([partition_name] if partition_name else [])
        donate = tuple(range(n_params, n_params + len(out_names)))

        def _body(*args):
            operands = list(args)
            if partition_name is not None:
                operands.append(bass2jax.partition_id_tensor())
            outs = bass2jax._bass_exec_p.bind(
                *operands, out_avals=tuple(out_avals), in_names=tuple(all_names),
                out_names=tuple(out_names), lowering_input_output_aliases=(),
                sim_require_finite=True, sim_require_nnan=True, nc=nc)
            return tuple(outs)

        devices = jax.devices()[:NC]
        mesh = Mesh(np.asarray(devices), ("core",))
        specs = (PartitionSpec("core"),) * (n_params + len(out_names))
        sharded = jax.jit(shard_map(_body, mesh=mesh, in_specs=specs,
                                    out_specs=(PartitionSpec("core"),) * len(out_names),
                                    check_rep=False),
                          donate_argnums=donate, keep_unused=True)
        _CACHE["jit"] = (sharded, in_names, out_names, out_avals)
    sharded, in_names, out_names, out_avals = _CACHE["jit"]
    concat_in = [np.concatenate([m[n] for m in maps], 0) for n in in_names]
    concat_zeros = [np.zeros((NC * a.shape[0], *a.shape[1:]), a.dtype) for a in out_avals]
    out_arrs = sharded(*concat_in, *concat_zeros)
    return [{n: np.asarray(out_arrs[i]).reshape(NC, *out_avals[i].shape)[c]
             for i, n in enumerate(out_names)} for c in range(NC)]


def kernel(input_tensor, qkv_kernel, gamma_qkv, beta_qkv, mean_qkv, var_qkv,
           query_rpe_table, key_rpe_table, value_rpe_table,
           gamma_sim, beta_sim, mean_sim, var_sim,
           gamma_out, beta_out, mean_out, var_out):
    import time
    inputs = dict(input_tensor=input_tensor, qkv_kernel=qkv_kernel,
                  gamma_qkv=gamma_qkv, beta_qkv=beta_qkv, mean_qkv=mean_qkv, var_qkv=var_qkv,
                  query_rpe_table=query_rpe_table, key_rpe_table=key_rpe_table,
                  value_rpe_table=value_rpe_table, gamma_sim=gamma_sim, beta_sim=beta_sim,
                  mean_sim=mean_sim, var_sim=var_sim, gamma_out=gamma_out, beta_out=beta_out,
                  mean_out=mean_out, var_out=var_out)
    P = _host_prep(inputs)
    if "nc" not in _CACHE:
        _CACHE["nc"] = _build()
    nc = _CACHE["nc"]
    x = np.asarray(input_tensor)
    B = x.shape[0]
    x_pad = np.zeros((NC * NI, L, C), np.float16)
    x_pad[:B] = x.astype(np.float16)
    maps = []
    for c in range(NC):
        m = {"x": x_pad[c * NI:(c + 1) * NI]}
        m.update({k: P[k] for k in ("wqk", "wv", "tqk", "qktab", "vt0", "vt1", "vt2", "tp")})
        maps.append(m)
    t0 = time.time()
    results = _run_cached(nc, maps)
    t1 = time.time()
    global LAST_HW_EXEC_NS
    LAST_HW_EXEC_NS = int((t1 - t0) * 1e9)
    out = np.concatenate([results[c]["out"] for c in range(NC)], 0)[:B]
    return out.astype(np.float32)
